# revision 18
# baseline (speedup 1.0000x reference)
"""Trainium2 Bass kernel for the NRI decoder (gnn_message_passing).

Strategy: data-parallel over batch B=8 across the 8 NeuronCores (one batch
item per core, params replicated; zero collectives).

Per-core algorithm (per recurrent step t, 9 steps):
  - fc1 of the edge MLP is factored through the nodes (exact, by
    associativity): pre @ W1 = rel_rec @ (hidden @ W1r) + rel_send @ (hidden @ W1s)
    so the heavy per-edge fc1 matmul collapses to two node-level matmuls
    (A = hidden@W1r, B = hidden@W1s) plus gather matmuls with rel_rec/rel_send.
  - gathers, fc2 and the scatter-aggregate are plain matmuls (no one-hot
    assumption anywhere), with the edge-type weights rtyp/(norm*d) folded into
    precomputed scatter weights wrec_k = rel_rec * rel_type[:, k] / 12 so the
    weighted sum over edge types becomes PSUM accumulation.
  - feature-major layouts chain all matmuls without transposes except one
    tiny [48,256] transpose of the aggregate per step.

Self-contained: hardcodes all shapes; no file reads.
"""

import numpy as np
import ml_dtypes

import concourse.bass as bass
import concourse.tile as tile
from concourse import bacc, mybir
from concourse.bass import ts
from concourse.bass_utils import run_bass_kernel_spmd
from concourse.masks import make_identity

# Problem constants
B, N, T, D, H, K = 8, 48, 10, 4, 256, 4
E = N * (N - 1)          # 2256
NK = K - 1               # 3 used edge types (type 0 skipped)
TS = T - 1               # 9 recurrent steps
NORM = float(NK * D)     # combined 1/(K-1) and 1/n_in_node scaling

F32 = mybir.dt.float32
F32R = mybir.dt.float32r
BF16 = mybir.dt.bfloat16
AF = mybir.ActivationFunctionType
ALU = mybir.AluOpType

NEC128 = (E + 127) // 128                                   # 18
EC128 = [(i * 128, min(128, E - i * 128)) for i in range(NEC128)]
EC512 = [(i * 512, min(512, E - i * 512)) for i in range((E + 511) // 512)]


def r32(ap):
    return ap.bitcast(F32R)


def build_decoder(skip_t0: bool) -> bass.Bass:
    nc = bacc.Bacc("TRN2", target_bir_lowering=False)

    d_data = nc.dram_tensor("data_fm", [5, T, N], F32, kind="ExternalInput")
    d_data_bf = nc.dram_tensor("data_bf", [5, T, N], BF16, kind="ExternalInput")
    d_relT = nc.dram_tensor("relT", [128, E], BF16, kind="ExternalInput")
    d_w1 = nc.dram_tensor("w1cat", [128, 2, NK, 2 * H], BF16, kind="ExternalInput")
    d_b1r = nc.dram_tensor("b1rows", [NK, H], BF16, kind="ExternalInput")
    d_w2 = nc.dram_tensor("w2", [128, 2, NK, H], BF16, kind="ExternalInput")
    d_b2 = nc.dram_tensor("b2bc", [128, NK, 2, H], BF16, kind="ExternalInput")
    d_wrec = nc.dram_tensor("wrec", [128, NK, NEC128, N], BF16, kind="ExternalInput")
    d_ghw = nc.dram_tensor("gru_hw", [128, 2, 3, H], BF16, kind="ExternalInput")
    d_giw = nc.dram_tensor("gru_iw", [5, 3, H], BF16, kind="ExternalInput")
    d_ow12 = nc.dram_tensor("outw12", [128, 2, 2, H], BF16, kind="ExternalInput")
    d_ob12 = nc.dram_tensor("outb12", [128, 2, 2], F32, kind="ExternalInput")
    d_o3w = nc.dram_tensor("out3w", [128, 2, D], BF16, kind="ExternalInput")
    d_o3b = nc.dram_tensor("out3b", [D, 1], F32, kind="ExternalInput")
    d_out = nc.dram_tensor("preds", [D, TS, N], F32, kind="ExternalOutput")

    with tile.TileContext(nc) as tc:
        with (
            tc.tile_pool(name="const", bufs=1) as cpool,
            tc.tile_pool(name="state", bufs=1) as spool,
            tc.tile_pool(name="work", bufs=3) as wpool,
            tc.tile_pool(name="stage", bufs=1) as zpool,
            tc.tile_pool(name="pA", bufs=1, space="PSUM") as pA,
            tc.tile_pool(name="pB", bufs=3, space="PSUM") as pB,
            tc.tile_pool(name="pC", bufs=1, space="PSUM") as pC,
        ):
            # ---------------- constants ----------------
            # Load order = consumption order: step-0 GRU inputs first so
            # compute starts immediately; big edge-phase tensors stream in
            # behind on a second DMA queue (gpsimd).
            ins5 = spool.tile([5, T, N], F32)
            nc.sync.dma_start(ins5[:], d_data[:])
            ins5b = spool.tile([5, T, N], BF16)
            nc.sync.dma_start(ins5b[:], d_data_bf[:])
            ghw = cpool.tile([128, 2, 3, H], BF16)
            nc.sync.dma_start(ghw[:], d_ghw[:])
            giw = cpool.tile([5, 3, H], BF16)
            nc.sync.dma_start(giw[:], d_giw[:])
            ow12 = cpool.tile([128, 2, 2, H], BF16)
            nc.sync.dma_start(ow12[:], d_ow12[:])
            ob12 = cpool.tile([128, 2, 2], F32)
            nc.sync.dma_start(ob12[:], d_ob12[:])
            o3w = cpool.tile([128, 2, D], BF16)
            nc.sync.dma_start(o3w[:], d_o3w[:])
            o3b = cpool.tile([D, 1], F32)
            nc.sync.dma_start(o3b[:], d_o3b[:])
            w1 = cpool.tile([128, 2, NK, 2 * H], BF16)
            nc.gpsimd.dma_start(w1[:], d_w1[:])
            relT = cpool.tile([128, E], BF16)
            nc.sync.dma_start(relT[:], d_relT[:])
            w2 = cpool.tile([128, 2, NK, H], BF16)
            nc.gpsimd.dma_start(w2[:], d_w2[:])
            b2bc = cpool.tile([128, NK, 2, H], BF16)
            nc.gpsimd.dma_start(b2bc[:], d_b2[:])
            wrec = cpool.tile([128, NK, NEC128, N], BF16)
            nc.gpsimd.dma_start(wrec[:], d_wrec[:])
            ident = cpool.tile([128, 128], F32)
            make_identity(nc, ident[:])

            # ---------------- state ----------------
            hidden = spool.tile([128, 2, N], BF16)
            nc.vector.memset(hidden[:], 0.0)
            agg = spool.tile([128, 2, N], BF16)
            nc.vector.memset(agg[:], 0.0)
            preds = spool.tile([D, TS, N], F32)
            m1 = spool.tile([128, 2, NK, E], BF16)
            # AB_k: rows 0:48 = A = hidden@W1r (node-major), rows 64:112 = B.
            # Rows 48:64 / 112:128 stay zero forever (they hit zero rows of relT).
            ABs = []
            for k in range(NK):
                ab = spool.tile([128, H], BF16, tag=f"AB{k}")
                nc.vector.memset(ab[:], 0.0)
                nc.sync.dma_start(ab[48:49, :], d_b1r[k : k + 1, :])
                ABs.append(ab)

            def node_fc1(k):
                ps = pB.tile([N, 2 * H], F32, tag="ps")
                for fc in range(2):
                    nc.tensor.matmul(
                        ps[:],
                        hidden[:, fc, :],
                        w1[:, fc, k, :],
                        start=(fc == 0),
                        stop=(fc == 1),
                    )
                nc.vector.tensor_copy(ABs[k][0:N, :], ps[:, 0:H])
                nc.vector.tensor_copy(ABs[k][64 : 64 + N, :], ps[:, H : 2 * H])

            def gather(k):
                # m1[h', e] = tanh(A[recv] + B[send] + b1): one matmul on the
                # stacked [A;B] / [rel_rec.T; rel_send.T] operands; the fc1
                # bias rides contract-row 48 (relT row 48 is all-ones, AB row
                # 48 holds b1), so the tanh needs no bias operand.
                for half in range(2):
                    for e0 in range(0, 2048, 1024):
                        ps = pA.tile([128, 1024], F32, tag="gather", bufs=2)
                        for c0 in range(0, 1024, 512):
                            nc.tensor.matmul(
                                ps[:, c0 : c0 + 512],
                                ABs[k][:, ts(half, 128)],
                                relT[:, e0 + c0 : e0 + c0 + 512],
                            )
                        nc.scalar.activation(
                            m1[:, half, k, e0 : e0 + 1024], ps[:], AF.Tanh
                        )
                # both halves' 208-col tails share one psum + one ACT
                pt = pA.tile([128, 2, E - 2048], F32, tag="gather", name="ptail", bufs=2)
                for half in range(2):
                    nc.tensor.matmul(
                        pt[:, half, :],
                        ABs[k][:, ts(half, 128)],
                        relT[:, 2048:E],
                    )
                nc.scalar.activation(m1[:, 0:2, k, 2048:E], pt[:], AF.Tanh)

            def fc2(k, z2):
                for g0 in range(0, NEC128, 2):
                    sub = EC128[g0 : g0 + 2]
                    ps = pB.tile([128, 2, H], F32, tag="ps")
                    for j, (e0, ew) in enumerate(sub):
                        for fc in range(2):
                            nc.tensor.matmul(
                                ps[:ew, j, :],
                                m1[:, fc, k, e0 : e0 + ew],
                                w2[:, fc, k, :],
                                start=(fc == 0),
                                stop=(fc == 1),
                            )
                    if sub[-1][1] == 128:
                        nc.vector.tensor_tensor(
                            z2[:, g0 : g0 + 2, :], ps[:], b2bc[:, k, :, :], ALU.add
                        )
                    else:
                        for j, (e0, ew) in enumerate(sub):
                            nc.vector.tensor_tensor(
                                z2[:ew, g0 + j, :],
                                ps[:ew, j, :],
                                b2bc[:ew, k, j, :],
                                ALU.add,
                            )

            TANH_PARTS = [(0, 8), (8, 17), (17, NEC128)]

            def mega_tanh(k, z2, m2, parts=None):
                if parts is None:
                    parts = TANH_PARTS
                for c0, c1 in parts:
                    if c1 == NEC128:
                        lw = EC128[-1][1]
                        if c1 - c0 > 1:
                            nc.scalar.activation(
                                m2[:, c0 : c1 - 1, :], z2[:, c0 : c1 - 1, :], AF.Tanh
                            )
                        nc.scalar.activation(
                            m2[:lw, c1 - 1, :], z2[:lw, c1 - 1, :], AF.Tanh
                        )
                    else:
                        nc.scalar.activation(m2[:, c0:c1, :], z2[:, c0:c1, :], AF.Tanh)

            def scatter(k, m2, agg_ps, first, last, chunks=(0, NEC128)):
                # col-packed pairs: even chunks -> psum rows 0:48 (col grp 0),
                # odd chunks -> rows 64:112 (col grp 64); pairs run concurrently.
                for ci in range(chunks[0], chunks[1]):
                    e0, ew = EC128[ci]
                    p = 64 * (ci % 2)
                    nc.tensor.matmul(
                        agg_ps[p : p + N, :],
                        wrec[:ew, k, ci, :],
                        m2[:ew, ci, :],
                        start=(first and ci in (0, 1)),
                        stop=(last and ci >= NEC128 - 2),
                        tile_position=(0, p),
                        skip_group_check=True,
                    )

            def edge_phase(pending_out=None):
                node_fc1(0)
                gather(0)
                if pending_out is not None:
                    pending_out()
                z2s = [zpool.tile([128, NEC128, H], F32, tag=f"z2_{k}", name=f"z2_{k}") for k in range(NK)]
                m2s = [zpool.tile([128, NEC128, H], BF16, tag=f"m2_{k}", name=f"m2_{k}") for k in range(NK)]
                agg_ps = pC.tile([128, H], F32, tag="agg")
                # software pipeline across edge types: PE keeps streaming while
                # the ACT mega-tanh of the previous type runs.
                node_fc1(1)
                gather(1)
                fc2(0, z2s[0])
                node_fc1(2)
                gather(2)
                mega_tanh(0, z2s[0], m2s[0])
                fc2(1, z2s[1])
                scatter(0, m2s[0], agg_ps, True, False)
                mega_tanh(1, z2s[1], m2s[1])
                fc2(2, z2s[2])
                scatter(1, m2s[1], agg_ps, False, False)
                # last type: interleave tanh parts with scatter parts so the
                # PE has work while the final tanh runs
                for pi, (c0, c1) in enumerate(TANH_PARTS):
                    mega_tanh(2, z2s[2], m2s[2], parts=((c0, c1),))
                    scatter(2, m2s[2], agg_ps, False, pi == len(TANH_PARTS) - 1,
                            chunks=(c0, c1))

                # agg_nm = rows[0:48] + rows[64:112]; then transpose to fm
                agg_nm = wpool.tile([N, H], F32, tag="aggnm")
                nc.vector.tensor_copy(agg_nm[:], agg_ps[0:N, :])
                nc.vector.tensor_tensor(
                    agg_nm[:], agg_nm[:], agg_ps[64 : 64 + N, :], ALU.add
                )
                for half in range(2):
                    tp = pB.tile([128, N], F32, tag="ps")
                    nc.tensor.transpose(tp[:], agg_nm[:, ts(half, 128)], ident[:N, :N])
                    nc.vector.tensor_copy(agg[:, half, :], tp[:])

            def gru_and_out(t):
                insT = ins5b[:, t, :]  # [5, 48]; row 4 is ones (folds input biases)
                gates = []
                for g in range(2):  # r, i
                    ps = pB.tile([128, 2, N], F32, tag="ps")
                    for half in range(2):
                        nc.tensor.matmul(
                            ps[:, half, :],
                            giw[:, g, ts(half, 128)],
                            insT,
                            start=True,
                            stop=False,
                        )
                        for fc in range(2):
                            nc.tensor.matmul(
                                ps[:, half, :],
                                ghw[:, fc, g, ts(half, 128)],
                                agg[:, fc, :],
                                start=False,
                                stop=(fc == 1),
                            )
                    gfm = wpool.tile([128, 2, N], F32, tag=f"gate{g}")
                    nc.scalar.activation(gfm[:], ps[:], AF.Sigmoid)
                    gates.append(gfm)
                r_fm, i_fm = gates

                ps_hn = pB.tile([128, 2, N], F32, tag="ps")
                ps_in = pB.tile([128, 2, N], F32, tag="ps")
                for half in range(2):
                    for fc in range(2):
                        nc.tensor.matmul(
                            ps_hn[:, half, :],
                            ghw[:, fc, 2, ts(half, 128)],
                            agg[:, fc, :],
                            start=(fc == 0),
                            stop=(fc == 1),
                        )
                    nc.tensor.matmul(
                        ps_in[:, half, :], giw[:, 2, ts(half, 128)], insT
                    )
                ka = pA.tile([128, N], F32, tag="gather", name="keepalive", bufs=2)
                nc.tensor.matmul(ka[0:2, :], r_fm[:, 0, 0:2], r_fm[:, 0, :])
                nc.tensor.matmul(ka[0:2, :], i_fm[:, 0, 0:2], i_fm[:, 0, :])
                t1 = wpool.tile([128, 2, N], F32, tag="t1")
                nng = wpool.tile([128, 2, N], F32, tag="nng")
                dlt = wpool.tile([128, 2, N], F32, tag="dlt")
                # per feature-half so hidb[:, h] lands early and the next
                # step's node-fc1 matmul can start sooner
                for h in range(2):
                    nc.vector.tensor_mul(t1[:, h, :], r_fm[:, h, :], ps_hn[:, h, :])
                    nc.vector.tensor_add(t1[:, h, :], t1[:, h, :], ps_in[:, h, :])
                    nc.scalar.activation(nng[:, h, :], t1[:, h, :], AF.Tanh)
                    # hidden = (1-i)*nng + i*hidden = nng + i*(hidden-nng)
                    nc.vector.tensor_sub(dlt[:, h, :], hidden[:, h, :], nng[:, h, :])
                    nc.vector.tensor_mul(dlt[:, h, :], i_fm[:, h, :], dlt[:, h, :])
                    nc.vector.tensor_add(hidden[:, h, :], nng[:, h, :], dlt[:, h, :])
                nc.tensor.matmul(ka[0:2, :], nng[:, 0, 0:2], nng[:, 0, :])

                def out_mlp(t=t):
                    emit_out_mlp(t)
                return out_mlp

            def emit_out_mlp(t):
                # output MLP with residual
                cur = hidden
                for layer in range(2):
                    ps = pB.tile([128, 2, N], F32, tag="ps")
                    for half in range(2):
                        for fc in range(2):
                            nc.tensor.matmul(
                                ps[:, half, :],
                                ow12[:, fc, layer, ts(half, 128)],
                                cur[:, fc, :],
                                start=(fc == 0),
                                stop=(fc == 1),
                            )
                    nxt = wpool.tile([128, 2, N], BF16, tag=f"p{layer}")
                    for half in range(2):
                        nc.vector.tensor_scalar(
                            nxt[:, half, :],
                            ps[:, half, :],
                            ob12[:, half, layer : layer + 1],
                            0.0,
                            ALU.add,
                            ALU.max,
                        )
                    cur = nxt
                ps3 = pB.tile([D, N], F32, tag="ps")
                for fc in range(2):
                    nc.tensor.matmul(
                        ps3[:],
                        o3w[:, fc, :],
                        cur[:, fc, :],
                        start=(fc == 0),
                        stop=(fc == 1),
                    )
                # pred = (ps3 + b3) + ins
                nc.vector.scalar_tensor_tensor(
                    preds[:, t, :], ps3[:], o3b[:], ins5[0:D, t, :], ALU.add, ALU.add
                )
                nc.sync.dma_start(d_out[:, t, :], preds[:, t, :])

            pending_out = None
            for t in range(TS):
                if not (skip_t0 and t == 0):
                    edge_phase(pending_out)
                    pending_out = None
                elif pending_out is not None:
                    pending_out()
                    pending_out = None
                pending_out = gru_and_out(t)
            pending_out()

    return nc


def _prep_core(b: int, inputs: dict) -> dict:
    f32 = np.float32
    bf16 = ml_dtypes.bfloat16
    data = np.asarray(inputs["data"], f32)
    rel_type = np.asarray(inputs["rel_type"], f32)
    rel_rec = np.asarray(inputs["rel_rec"], f32)
    rel_send = np.asarray(inputs["rel_send"], f32)
    w1 = np.asarray(inputs["msg_fc1_w"], f32)
    b1 = np.asarray(inputs["msg_fc1_b"], f32)
    w2 = np.asarray(inputs["msg_fc2_w"], f32)
    b2 = np.asarray(inputs["msg_fc2_b"], f32)

    m = {}
    dfm = np.ones((5, T, N), f32)
    dfm[0:4] = data[b].transpose(2, 1, 0)  # [N,T,D] -> [D,T,N]
    m["data_fm"] = dfm
    m["data_bf"] = dfm.astype(bf16)

    relT = np.zeros((128, E), f32)
    relT[0:N] = rel_rec.T
    relT[48] = 1.0  # bias row: pairs with AB row 48 = msg_fc1_b
    relT[64 : 64 + N] = rel_send.T
    m["relT"] = relT.astype(bf16)

    w1c = np.zeros((128, 2, NK, 2 * H), f32)
    for k in range(NK):
        wk = w1[k + 1]  # [2H, H]
        cat = np.concatenate([wk[:H], wk[H:]], axis=1)  # [H, 2H] = [W1r | W1s]
        w1c[:, :, k, :] = cat.reshape(2, 128, 2 * H).transpose(1, 0, 2)
    m["w1cat"] = w1c.astype(bf16)
    m["b1rows"] = np.stack([b1[k + 1] for k in range(NK)], axis=0).astype(bf16)

    m["w2"] = np.stack(
        [w2[k + 1].reshape(2, 128, H).transpose(1, 0, 2) for k in range(NK)], axis=2
    ).astype(bf16)
    b2bc = np.zeros((128, NK, 2, H), f32)
    for k in range(NK):
        b2bc[:, k, :, :] = b2[k + 1][None, None, :]
    m["b2bc"] = b2bc.astype(bf16)

    wr = np.zeros((128, NK, NEC128, N), f32)
    for k in range(NK):
        wk = rel_rec * rel_type[b, :, k + 1 : k + 2] / NORM  # [E, N]
        wkp = np.zeros((NEC128 * 128, N), f32)
        wkp[:E] = wk
        wr[:, k] = wkp.reshape(NEC128, 128, N).transpose(1, 0, 2)
    m["wrec"] = wr.astype(bf16)

    m["gru_hw"] = np.stack(
        [
            np.asarray(inputs[n], f32).reshape(2, 128, H).transpose(1, 0, 2)
            for n in ["hid_r_w", "hid_i_w", "hid_n_w"]
        ],
        axis=2,
    ).astype(bf16)
    giw = np.zeros((5, 3, H), f32)
    for g, (wn, bn) in enumerate(
        [("in_r_w", "in_r_b"), ("in_i_w", "in_i_b"), ("in_n_w", "in_n_b")]
    ):
        giw[0:4, g] = np.asarray(inputs[wn], f32)
        giw[4, g] = np.asarray(inputs[bn], f32)
    m["gru_iw"] = giw.astype(bf16)

    m["outw12"] = np.stack(
        [
            np.asarray(inputs[n], f32).reshape(2, 128, H).transpose(1, 0, 2)
            for n in ["out1_w", "out2_w"]
        ],
        axis=2,
    ).astype(bf16)
    m["outb12"] = np.stack(
        [np.asarray(inputs[n], f32).reshape(2, 128).T for n in ["out1_b", "out2_b"]],
        axis=2,
    )
    m["out3w"] = np.asarray(inputs["out3_w"], f32).reshape(2, 128, D).transpose(1, 0, 2).astype(bf16)
    m["out3b"] = np.asarray(inputs["out3_b"], f32).reshape(D, 1)
    return m


def _skip_t0_ok(inputs) -> bool:
    # With hidden0 == 0, the whole edge phase at t=0 yields agg == 0 iff the
    # message-MLP biases of the used edge types are zero.
    return bool(
        np.all(np.asarray(inputs["msg_fc1_b"])[1:] == 0)
        and np.all(np.asarray(inputs["msg_fc2_b"])[1:] == 0)
    )


def kernel(**inputs) -> np.ndarray:
    assert int(inputs["pred_steps"]) == 1
    skip_t0 = _skip_t0_ok(inputs)
    nc = build_decoder(skip_t0)
    nc.compile()
    in_maps = [_prep_core(b, inputs) for b in range(B)]
    res = run_bass_kernel_spmd(nc, in_maps, core_ids=list(range(B)))
    out = np.stack(
        [res.results[b]["preds"].transpose(2, 1, 0) for b in range(B)], axis=0
    )
    return out.astype(np.float32)


if __name__ == "__main__":
    # smoke: build only
    nc = build_decoder(True)
    print("built ok")


# revision 19
# speedup vs baseline: 1.0372x; 1.0372x over previous
"""Trainium2 Bass kernel for the NRI decoder (gnn_message_passing).

Strategy: data-parallel over batch B=8 across the 8 NeuronCores (one batch
item per core, params replicated; zero collectives).

Per-core algorithm (per recurrent step t, 9 steps):
  - fc1 of the edge MLP is factored through the nodes (exact, by
    associativity): pre @ W1 = rel_rec @ (hidden @ W1r) + rel_send @ (hidden @ W1s)
    so the heavy per-edge fc1 matmul collapses to two node-level matmuls
    (A = hidden@W1r, B = hidden@W1s) plus gather matmuls with rel_rec/rel_send.
  - gathers, fc2 and the scatter-aggregate are plain matmuls (no one-hot
    assumption anywhere), with the edge-type weights rtyp/(norm*d) folded into
    precomputed scatter weights wrec_k = rel_rec * rel_type[:, k] / 12 so the
    weighted sum over edge types becomes PSUM accumulation.
  - feature-major layouts chain all matmuls without transposes except one
    tiny [48,256] transpose of the aggregate per step.

Self-contained: hardcodes all shapes; no file reads.
"""

import numpy as np
import ml_dtypes

import concourse.bass as bass
import concourse.tile as tile
from concourse import bacc, mybir
from concourse.bass import ts
from concourse.bass_utils import run_bass_kernel_spmd
from concourse.masks import make_identity

# Problem constants
B, N, T, D, H, K = 8, 48, 10, 4, 256, 4
E = N * (N - 1)          # 2256
NK = K - 1               # 3 used edge types (type 0 skipped)
TS = T - 1               # 9 recurrent steps
NORM = float(NK * D)     # combined 1/(K-1) and 1/n_in_node scaling

F32 = mybir.dt.float32
F32R = mybir.dt.float32r
BF16 = mybir.dt.bfloat16
AF = mybir.ActivationFunctionType
ALU = mybir.AluOpType

NEC128 = (E + 127) // 128                                   # 18
EC128 = [(i * 128, min(128, E - i * 128)) for i in range(NEC128)]
EC512 = [(i * 512, min(512, E - i * 512)) for i in range((E + 511) // 512)]


def r32(ap):
    return ap.bitcast(F32R)


def build_decoder(skip_t0: bool) -> bass.Bass:
    nc = bacc.Bacc("TRN2", target_bir_lowering=False)

    d_data = nc.dram_tensor("data_fm", [5, T, N], F32, kind="ExternalInput")
    d_data_bf = nc.dram_tensor("data_bf", [5, T, N], BF16, kind="ExternalInput")
    d_relT = nc.dram_tensor("relT", [128, E], BF16, kind="ExternalInput")
    d_w1 = nc.dram_tensor("w1cat", [128, 2, NK, 2 * H], BF16, kind="ExternalInput")
    d_b1r = nc.dram_tensor("b1rows", [NK, H], BF16, kind="ExternalInput")
    d_w2 = nc.dram_tensor("w2", [128, 2, NK, H], BF16, kind="ExternalInput")
    d_b2 = nc.dram_tensor("b2bc", [128, NK, 2, H], BF16, kind="ExternalInput")
    d_wrec = nc.dram_tensor("wrec", [128, NK, NEC128, N], BF16, kind="ExternalInput")
    d_ghw = nc.dram_tensor("gru_hw", [128, 2, 3, H], BF16, kind="ExternalInput")
    d_giw = nc.dram_tensor("gru_iw", [5, 3, H], BF16, kind="ExternalInput")
    d_ow12 = nc.dram_tensor("outw12", [128, 2, 2, H], BF16, kind="ExternalInput")
    d_ob12 = nc.dram_tensor("outb12", [128, 2, 2], F32, kind="ExternalInput")
    d_o3w = nc.dram_tensor("out3w", [128, 2, D], BF16, kind="ExternalInput")
    d_o3b = nc.dram_tensor("out3b", [D, 1], F32, kind="ExternalInput")
    d_out = nc.dram_tensor("preds", [D, TS, N], F32, kind="ExternalOutput")

    with tile.TileContext(nc) as tc:
        with (
            tc.tile_pool(name="const", bufs=1) as cpool,
            tc.tile_pool(name="state", bufs=1) as spool,
            tc.tile_pool(name="work", bufs=3) as wpool,
            tc.tile_pool(name="stage", bufs=1) as zpool,
            tc.tile_pool(name="pA", bufs=1, space="PSUM") as pA,
            tc.tile_pool(name="pB", bufs=3, space="PSUM") as pB,
            tc.tile_pool(name="pC", bufs=1, space="PSUM") as pC,
        ):
            # ---------------- constants ----------------
            # Load order = consumption order: step-0 GRU inputs first so
            # compute starts immediately; big edge-phase tensors stream in
            # behind on a second DMA queue (gpsimd).
            ins5 = spool.tile([5, T, N], F32)
            nc.sync.dma_start(ins5[:], d_data[:])
            ins5b = spool.tile([5, T, N], BF16)
            nc.sync.dma_start(ins5b[:], d_data_bf[:])
            ghw = cpool.tile([128, 2, 3, H], BF16)
            nc.sync.dma_start(ghw[:], d_ghw[:])
            giw = cpool.tile([5, 3, H], BF16)
            nc.sync.dma_start(giw[:], d_giw[:])
            ow12 = cpool.tile([128, 2, 2, H], BF16)
            nc.sync.dma_start(ow12[:], d_ow12[:])
            ob12 = cpool.tile([128, 2, 2], F32)
            nc.sync.dma_start(ob12[:], d_ob12[:])
            o3w = cpool.tile([128, 2, D], BF16)
            nc.sync.dma_start(o3w[:], d_o3w[:])
            o3b = cpool.tile([D, 1], F32)
            nc.sync.dma_start(o3b[:], d_o3b[:])
            w1 = cpool.tile([128, 2, NK, 2 * H], BF16)
            nc.gpsimd.dma_start(w1[:], d_w1[:])
            relT = cpool.tile([128, E], BF16)
            nc.sync.dma_start(relT[:], d_relT[:])
            w2 = cpool.tile([128, 2, NK, H], BF16)
            nc.gpsimd.dma_start(w2[:], d_w2[:])
            b2bc = cpool.tile([128, NK, 2, H], BF16)
            nc.gpsimd.dma_start(b2bc[:], d_b2[:])
            wrec = cpool.tile([128, NK, NEC128, N], BF16)
            nc.gpsimd.dma_start(wrec[:], d_wrec[:])
            ident = cpool.tile([128, 128], F32)
            make_identity(nc, ident[:])

            # ---------------- state ----------------
            hidden = spool.tile([128, 2, N], BF16)
            nc.vector.memset(hidden[:], 0.0)
            agg = spool.tile([128, 2, N], BF16)
            nc.vector.memset(agg[:], 0.0)
            preds = spool.tile([D, TS, N], F32)
            m1 = spool.tile([128, 2, NK, E], BF16)
            # AB_k: rows 0:48 = A = hidden@W1r (node-major), rows 64:112 = B.
            # Rows 48:64 / 112:128 stay zero forever (they hit zero rows of relT).
            ABs = []
            for k in range(NK):
                ab = spool.tile([128, H], BF16, tag=f"AB{k}")
                nc.vector.memset(ab[:], 0.0)
                nc.sync.dma_start(ab[48:49, :], d_b1r[k : k + 1, :])
                ABs.append(ab)

            def node_fc1(k):
                ps = pB.tile([N, 2 * H], F32, tag="ps")
                for fc in range(2):
                    nc.tensor.matmul(
                        ps[:],
                        hidden[:, fc, :],
                        w1[:, fc, k, :],
                        start=(fc == 0),
                        stop=(fc == 1),
                    )
                nc.vector.tensor_copy(ABs[k][0:N, :], ps[:, 0:H])
                nc.vector.tensor_copy(ABs[k][64 : 64 + N, :], ps[:, H : 2 * H])

            def gather(k):
                # m1[h', e] = tanh(A[recv] + B[send] + b1): one matmul on the
                # stacked [A;B] / [rel_rec.T; rel_send.T] operands; the fc1
                # bias rides contract-row 48 (relT row 48 is all-ones, AB row
                # 48 holds b1), so the tanh needs no bias operand.
                for half in range(2):
                    for e0 in range(0, 2048, 1024):
                        ps = pA.tile([128, 1024], F32, tag="gather", bufs=2)
                        for c0 in range(0, 1024, 512):
                            nc.tensor.matmul(
                                ps[:, c0 : c0 + 512],
                                ABs[k][:, ts(half, 128)],
                                relT[:, e0 + c0 : e0 + c0 + 512],
                            )
                        nc.scalar.activation(
                            m1[:, half, k, e0 : e0 + 1024], ps[:], AF.Tanh
                        )
                # both halves' 208-col tails share one psum + one ACT
                pt = pA.tile([128, 2, E - 2048], F32, tag="gather", name="ptail", bufs=2)
                for half in range(2):
                    nc.tensor.matmul(
                        pt[:, half, :],
                        ABs[k][:, ts(half, 128)],
                        relT[:, 2048:E],
                    )
                nc.scalar.activation(m1[:, 0:2, k, 2048:E], pt[:], AF.Tanh)

            def fc2(k, z2, groups=(0, NEC128)):
                for g0 in range(groups[0], groups[1], 2):
                    sub = EC128[g0 : g0 + 2]
                    ps = pB.tile([128, 2, H], F32, tag="ps")
                    for j, (e0, ew) in enumerate(sub):
                        for fc in range(2):
                            nc.tensor.matmul(
                                ps[:ew, j, :],
                                m1[:, fc, k, e0 : e0 + ew],
                                w2[:, fc, k, :],
                                start=(fc == 0),
                                stop=(fc == 1),
                            )
                    if sub[-1][1] == 128:
                        nc.vector.tensor_tensor(
                            z2[:, g0 : g0 + 2, :], ps[:], b2bc[:, k, :, :], ALU.add
                        )
                    else:
                        for j, (e0, ew) in enumerate(sub):
                            nc.vector.tensor_tensor(
                                z2[:ew, g0 + j, :],
                                ps[:ew, j, :],
                                b2bc[:ew, k, j, :],
                                ALU.add,
                            )

            TANH_PARTS = [(0, 8), (8, 17), (17, NEC128)]

            def mega_tanh(k, z2, m2, parts=None):
                if parts is None:
                    parts = TANH_PARTS
                for c0, c1 in parts:
                    if c1 == NEC128:
                        lw = EC128[-1][1]
                        if c1 - c0 > 1:
                            nc.scalar.activation(
                                m2[:, c0 : c1 - 1, :], z2[:, c0 : c1 - 1, :], AF.Tanh
                            )
                        nc.scalar.activation(
                            m2[:lw, c1 - 1, :], z2[:lw, c1 - 1, :], AF.Tanh
                        )
                    else:
                        nc.scalar.activation(m2[:, c0:c1, :], z2[:, c0:c1, :], AF.Tanh)

            def scatter(k, m2, agg_ps, first, last, chunks=(0, NEC128)):
                # col-packed pairs: even chunks -> psum rows 0:48 (col grp 0),
                # odd chunks -> rows 64:112 (col grp 64); pairs run concurrently.
                for ci in range(chunks[0], chunks[1]):
                    e0, ew = EC128[ci]
                    p = 64 * (ci % 2)
                    nc.tensor.matmul(
                        agg_ps[p : p + N, :],
                        wrec[:ew, k, ci, :],
                        m2[:ew, ci, :],
                        start=(first and ci in (0, 1)),
                        stop=(last and ci >= NEC128 - 2),
                        tile_position=(0, p),
                        skip_group_check=True,
                    )

            def edge_phase(pending_out=None):
                node_fc1(0)
                gather(0)
                z2s = [zpool.tile([128, NEC128, H], F32, tag=f"z2_{k}", name=f"z2_{k}") for k in range(NK)]
                m2s = [zpool.tile([128, NEC128, H], BF16, tag=f"m2_{k}", name=f"m2_{k}") for k in range(NK)]
                agg_ps = pC.tile([128, H], F32, tag="agg")
                # software pipeline across edge types: PE keeps streaming while
                # the ACT mega-tanh of the previous type runs; gathers lead so
                # the m1-tanh stream never starves.
                node_fc1(1)
                gather(1)
                fc2(0, z2s[0], (0, 8))
                node_fc1(2)
                gather(2)
                fc2(0, z2s[0], (8, NEC128))
                if pending_out is not None:
                    pending_out()
                mega_tanh(0, z2s[0], m2s[0])
                fc2(1, z2s[1])
                scatter(0, m2s[0], agg_ps, True, False)
                mega_tanh(1, z2s[1], m2s[1])
                fc2(2, z2s[2])
                scatter(1, m2s[1], agg_ps, False, False)
                # last type: interleave tanh parts with scatter parts so the
                # PE has work while the final tanh runs
                for pi, (c0, c1) in enumerate(TANH_PARTS):
                    mega_tanh(2, z2s[2], m2s[2], parts=((c0, c1),))
                    scatter(2, m2s[2], agg_ps, False, pi == len(TANH_PARTS) - 1,
                            chunks=(c0, c1))

                # agg_nm = rows[0:48] + rows[64:112]; then transpose to fm
                agg_nm = wpool.tile([N, H], F32, tag="aggnm")
                nc.vector.tensor_copy(agg_nm[:], agg_ps[0:N, :])
                nc.vector.tensor_tensor(
                    agg_nm[:], agg_nm[:], agg_ps[64 : 64 + N, :], ALU.add
                )
                for half in range(2):
                    tp = pB.tile([128, N], F32, tag="ps")
                    nc.tensor.transpose(tp[:], agg_nm[:, ts(half, 128)], ident[:N, :N])
                    nc.vector.tensor_copy(agg[:, half, :], tp[:])

            def gru_and_out(t):
                insT = ins5b[:, t, :]  # [5, 48]; row 4 is ones (folds input biases)
                gates = []
                for g in range(2):  # r, i
                    ps = pB.tile([128, 2, N], F32, tag="ps")
                    for half in range(2):
                        nc.tensor.matmul(
                            ps[:, half, :],
                            giw[:, g, ts(half, 128)],
                            insT,
                            start=True,
                            stop=False,
                        )
                        for fc in range(2):
                            nc.tensor.matmul(
                                ps[:, half, :],
                                ghw[:, fc, g, ts(half, 128)],
                                agg[:, fc, :],
                                start=False,
                                stop=(fc == 1),
                            )
                    gfm = wpool.tile([128, 2, N], F32, tag=f"gate{g}")
                    nc.scalar.activation(gfm[:], ps[:], AF.Sigmoid)
                    gates.append(gfm)
                r_fm, i_fm = gates

                ps_hn = pB.tile([128, 2, N], F32, tag="ps")
                ps_in = pB.tile([128, 2, N], F32, tag="ps")
                for half in range(2):
                    for fc in range(2):
                        nc.tensor.matmul(
                            ps_hn[:, half, :],
                            ghw[:, fc, 2, ts(half, 128)],
                            agg[:, fc, :],
                            start=(fc == 0),
                            stop=(fc == 1),
                        )
                    nc.tensor.matmul(
                        ps_in[:, half, :], giw[:, 2, ts(half, 128)], insT
                    )
                ka = pA.tile([128, N], F32, tag="gather", name="keepalive", bufs=2)
                nc.tensor.matmul(ka[0:2, :], r_fm[:, 0, 0:2], r_fm[:, 0, :])
                nc.tensor.matmul(ka[0:2, :], i_fm[:, 0, 0:2], i_fm[:, 0, :])
                t1 = wpool.tile([128, 2, N], F32, tag="t1")
                nng = wpool.tile([128, 2, N], F32, tag="nng")
                dlt = wpool.tile([128, 2, N], F32, tag="dlt")
                # per feature-half so hidb[:, h] lands early and the next
                # step's node-fc1 matmul can start sooner
                for h in range(2):
                    nc.vector.tensor_mul(t1[:, h, :], r_fm[:, h, :], ps_hn[:, h, :])
                    nc.vector.tensor_add(t1[:, h, :], t1[:, h, :], ps_in[:, h, :])
                    nc.scalar.activation(nng[:, h, :], t1[:, h, :], AF.Tanh)
                    # hidden = (1-i)*nng + i*hidden = nng + i*(hidden-nng)
                    nc.vector.tensor_sub(dlt[:, h, :], hidden[:, h, :], nng[:, h, :])
                    nc.vector.tensor_mul(dlt[:, h, :], i_fm[:, h, :], dlt[:, h, :])
                    nc.vector.tensor_add(hidden[:, h, :], nng[:, h, :], dlt[:, h, :])
                nc.tensor.matmul(ka[0:2, :], nng[:, 0, 0:2], nng[:, 0, :])

                def out_mlp(t=t):
                    emit_out_mlp(t)
                return out_mlp

            def emit_out_mlp(t):
                # output MLP with residual
                cur = hidden
                for layer in range(2):
                    ps = pB.tile([128, 2, N], F32, tag="ps")
                    for half in range(2):
                        for fc in range(2):
                            nc.tensor.matmul(
                                ps[:, half, :],
                                ow12[:, fc, layer, ts(half, 128)],
                                cur[:, fc, :],
                                start=(fc == 0),
                                stop=(fc == 1),
                            )
                    nxt = wpool.tile([128, 2, N], BF16, tag=f"p{layer}")
                    for half in range(2):
                        nc.vector.tensor_scalar(
                            nxt[:, half, :],
                            ps[:, half, :],
                            ob12[:, half, layer : layer + 1],
                            0.0,
                            ALU.add,
                            ALU.max,
                        )
                    cur = nxt
                ps3 = pB.tile([D, N], F32, tag="ps")
                for fc in range(2):
                    nc.tensor.matmul(
                        ps3[:],
                        o3w[:, fc, :],
                        cur[:, fc, :],
                        start=(fc == 0),
                        stop=(fc == 1),
                    )
                # pred = (ps3 + b3) + ins
                nc.vector.scalar_tensor_tensor(
                    preds[:, t, :], ps3[:], o3b[:], ins5[0:D, t, :], ALU.add, ALU.add
                )
                nc.sync.dma_start(d_out[:, t, :], preds[:, t, :])

            pending_out = None
            for t in range(TS):
                if not (skip_t0 and t == 0):
                    edge_phase(pending_out)
                    pending_out = None
                elif pending_out is not None:
                    pending_out()
                    pending_out = None
                pending_out = gru_and_out(t)
            pending_out()

    return nc


def _prep_core(b: int, inputs: dict) -> dict:
    f32 = np.float32
    bf16 = ml_dtypes.bfloat16
    data = np.asarray(inputs["data"], f32)
    rel_type = np.asarray(inputs["rel_type"], f32)
    rel_rec = np.asarray(inputs["rel_rec"], f32)
    rel_send = np.asarray(inputs["rel_send"], f32)
    w1 = np.asarray(inputs["msg_fc1_w"], f32)
    b1 = np.asarray(inputs["msg_fc1_b"], f32)
    w2 = np.asarray(inputs["msg_fc2_w"], f32)
    b2 = np.asarray(inputs["msg_fc2_b"], f32)

    m = {}
    dfm = np.ones((5, T, N), f32)
    dfm[0:4] = data[b].transpose(2, 1, 0)  # [N,T,D] -> [D,T,N]
    m["data_fm"] = dfm
    m["data_bf"] = dfm.astype(bf16)

    relT = np.zeros((128, E), f32)
    relT[0:N] = rel_rec.T
    relT[48] = 1.0  # bias row: pairs with AB row 48 = msg_fc1_b
    relT[64 : 64 + N] = rel_send.T
    m["relT"] = relT.astype(bf16)

    w1c = np.zeros((128, 2, NK, 2 * H), f32)
    for k in range(NK):
        wk = w1[k + 1]  # [2H, H]
        cat = np.concatenate([wk[:H], wk[H:]], axis=1)  # [H, 2H] = [W1r | W1s]
        w1c[:, :, k, :] = cat.reshape(2, 128, 2 * H).transpose(1, 0, 2)
    m["w1cat"] = w1c.astype(bf16)
    m["b1rows"] = np.stack([b1[k + 1] for k in range(NK)], axis=0).astype(bf16)

    m["w2"] = np.stack(
        [w2[k + 1].reshape(2, 128, H).transpose(1, 0, 2) for k in range(NK)], axis=2
    ).astype(bf16)
    b2bc = np.zeros((128, NK, 2, H), f32)
    for k in range(NK):
        b2bc[:, k, :, :] = b2[k + 1][None, None, :]
    m["b2bc"] = b2bc.astype(bf16)

    wr = np.zeros((128, NK, NEC128, N), f32)
    for k in range(NK):
        wk = rel_rec * rel_type[b, :, k + 1 : k + 2] / NORM  # [E, N]
        wkp = np.zeros((NEC128 * 128, N), f32)
        wkp[:E] = wk
        wr[:, k] = wkp.reshape(NEC128, 128, N).transpose(1, 0, 2)
    m["wrec"] = wr.astype(bf16)

    m["gru_hw"] = np.stack(
        [
            np.asarray(inputs[n], f32).reshape(2, 128, H).transpose(1, 0, 2)
            for n in ["hid_r_w", "hid_i_w", "hid_n_w"]
        ],
        axis=2,
    ).astype(bf16)
    giw = np.zeros((5, 3, H), f32)
    for g, (wn, bn) in enumerate(
        [("in_r_w", "in_r_b"), ("in_i_w", "in_i_b"), ("in_n_w", "in_n_b")]
    ):
        giw[0:4, g] = np.asarray(inputs[wn], f32)
        giw[4, g] = np.asarray(inputs[bn], f32)
    m["gru_iw"] = giw.astype(bf16)

    m["outw12"] = np.stack(
        [
            np.asarray(inputs[n], f32).reshape(2, 128, H).transpose(1, 0, 2)
            for n in ["out1_w", "out2_w"]
        ],
        axis=2,
    ).astype(bf16)
    m["outb12"] = np.stack(
        [np.asarray(inputs[n], f32).reshape(2, 128).T for n in ["out1_b", "out2_b"]],
        axis=2,
    )
    m["out3w"] = np.asarray(inputs["out3_w"], f32).reshape(2, 128, D).transpose(1, 0, 2).astype(bf16)
    m["out3b"] = np.asarray(inputs["out3_b"], f32).reshape(D, 1)
    return m


def _skip_t0_ok(inputs) -> bool:
    # With hidden0 == 0, the whole edge phase at t=0 yields agg == 0 iff the
    # message-MLP biases of the used edge types are zero.
    return bool(
        np.all(np.asarray(inputs["msg_fc1_b"])[1:] == 0)
        and np.all(np.asarray(inputs["msg_fc2_b"])[1:] == 0)
    )


def kernel(**inputs) -> np.ndarray:
    assert int(inputs["pred_steps"]) == 1
    skip_t0 = _skip_t0_ok(inputs)
    nc = build_decoder(skip_t0)
    nc.compile()
    in_maps = [_prep_core(b, inputs) for b in range(B)]
    res = run_bass_kernel_spmd(nc, in_maps, core_ids=list(range(B)))
    out = np.stack(
        [res.results[b]["preds"].transpose(2, 1, 0) for b in range(B)], axis=0
    )
    return out.astype(np.float32)


if __name__ == "__main__":
    # smoke: build only
    nc = build_decoder(True)
    print("built ok")
